# revision 20
# baseline (speedup 1.0000x reference)
"""Deformable-attention block kernel for Trainium2 (8 NeuronCores).

Sharding: data-parallel over (batch, image-half): core i handles image i//2,
rows [64*(i%2), 64*(i%2)+64).

Per-core pipeline (all compute on device):
  1. offset conv (3x3, PE f32, PSUM-accumulated shifted matmuls)
  2. per-pixel map math (DVE f32): sample coords, floor/frac, clamps, masks,
     bilinear corner weights, flat gather indices
  3. index relayout to dma_gather's wrapped [16, S] layout via PE transposes
  4. dma_gather from a padded row-pair image in HBM: ONE descriptor per
     (kernel-point, pixel) fetches the full 2x2 bilinear patch
  5. bilinear blend (DVE, per-partition weights with stride-0 broadcast)
  6. PE transpose + deform conv (9 matmuls accumulated in PSUM)
  7. bias + sigmoid attention gate + relu, pixel-major output
"""
import dataclasses
import sys

import numpy as np

sys.path.insert(0, "/opt/trn_rl_repo")

import concourse.bacc as bacc  # noqa: E402
import concourse.bass as bass  # noqa: E402
import concourse.mybir as mybir  # noqa: E402
import concourse.tile as tile  # noqa: E402
from concourse.bass_utils import run_bass_kernel_spmd  # noqa: E402
from concourse.library_config import mlp  # noqa: E402

F32 = mybir.dt.float32
I16 = mybir.dt.int16
AF = mybir.ActivationFunctionType
ALU = mybir.AluOpType

B, C, H, W = 4, 128, 128, 128
N_CORES = 8
HALF = 64          # output rows per core
PADR = 130         # padded coords: y,x in [-1,128]
NPAIR = PADR * PADR + 8   # pair-image rows (+ slack zeros for idx+1 overflow)
ELEM = 512         # f32 elements per gather descriptor (2 pair-entries)
ESTEP = 256        # f32 elements per pair-entry row
YB = 8             # output rows per gather chunk
NCHUNK = HALF // YB  # 8 chunks per k
GS = 1024          # idxs per gather instruction (8 rows x 128 px)

_cache = {}


def _bcast0(ap, n):
    """Append a stride-0 dim of size n to an AP (free-dim broadcast)."""
    return dataclasses.replace(ap, ap=list(ap.ap) + [[0, n]])


def _bcast_tile(t, n):
    """[128, N] tile -> [128, n, N] with stride-0 middle dim."""
    ap = t[:]
    return dataclasses.replace(
        ap, ap=[ap.ap[0], [0, n], ap.ap[1]])


def build_program(debug=False):
    nc = bacc.Bacc("TRN2", target_bir_lowering=False, debug=False,
                   num_devices=N_CORES)

    # ---- DRAM I/O ----
    pair_d = nc.dram_tensor("pair", [NPAIR * ESTEP], F32, kind="ExternalInput")
    pair_ap = dataclasses.replace(
        pair_d.ap(), ap=[[ESTEP, NPAIR - 1], [1, ELEM]])
    xc_d = nc.dram_tensor("xc", [128, 66 * PADR], F32, kind="ExternalInput")
    woff_d = nc.dram_tensor("woff", [128, 9 * 18], F32, kind="ExternalInput")
    boff_d = nc.dram_tensor("boff", [128, 18], F32, kind="ExternalInput")
    wdef_d = nc.dram_tensor("wdef", [128, 9 * 128], mybir.dt.bfloat16, kind="ExternalInput")
    bdef_d = nc.dram_tensor("bdef", [128, 128], F32, kind="ExternalInput")
    wattn_d = nc.dram_tensor("wattn", [128, 128], F32, kind="ExternalInput")
    battn_d = nc.dram_tensor("battn", [128, 1], F32, kind="ExternalInput")
    basey_d = nc.dram_tensor("basey", [128, 576], F32, kind="ExternalInput")
    basex_d = nc.dram_tensor("basex", [128, 576], F32, kind="ExternalInput")
    iota_d = nc.dram_tensor("iota", [128, 1], F32, kind="ExternalInput")
    ident_d = nc.dram_tensor("ident", [128, 128], F32, kind="ExternalInput")
    out_d = nc.dram_tensor("out", [128, HALF * 128], F32, kind="ExternalOutput")
    if debug:
        dbg_off_d = nc.dram_tensor("dbg_off", [128, 64 * 18], F32,
                                   kind="ExternalOutput")
        dbg_idx_d = nc.dram_tensor("dbg_idx", [128, 4608], I16,
                                   kind="ExternalOutput")
        dbg_w_d = nc.dram_tensor("dbg_w", [128, 4 * 576], F32,
                                 kind="ExternalOutput")
        dbg_s_d = nc.dram_tensor("dbg_s", [128, 9 * 128],
                                 mybir.dt.bfloat16, kind="ExternalOutput")

    gsems = [nc.alloc_semaphore(f"gsem{i}") for i in range(3)]

    with tile.TileContext(nc) as tc:
        import contextlib
        with contextlib.ExitStack() as ctx:
            cpool = ctx.enter_context(tc.tile_pool(name="consts", bufs=1))
            mpool = ctx.enter_context(tc.tile_pool(name="maps", bufs=1))
            tpool = ctx.enter_context(tc.tile_pool(name="tmp", bufs=1))
            spool = ctx.enter_context(tc.tile_pool(name="swork", bufs=3))
            psum = ctx.enter_context(
                tc.tile_pool(name="psum", bufs=2, space="PSUM"))
            opsum = ctx.enter_context(
                tc.tile_pool(name="opsum", bufs=4, space="PSUM"))

            # ---- load constants ----
            xc = cpool.tile([128, 66 * PADR], F32)
            nc.sync.dma_start(xc[:], xc_d.ap())
            woff = cpool.tile([128, 9 * 18], F32)
            nc.sync.dma_start(woff[:], woff_d.ap())
            boff = cpool.tile([128, 18], F32)
            nc.sync.dma_start(boff[:], boff_d.ap())
            wdef = cpool.tile([128, 9 * 128], mybir.dt.bfloat16)
            nc.sync.dma_start(wdef[:], wdef_d.ap())
            bdef = cpool.tile([128, 128], F32)
            nc.sync.dma_start(bdef[:], bdef_d.ap())
            bdef_rep4 = dataclasses.replace(
                bdef[:], ap=[bdef[:].ap[0], [0, 4], bdef[:].ap[1]])
            wattn = cpool.tile([128, 128], F32)
            nc.sync.dma_start(wattn[:], wattn_d.ap())
            battn = cpool.tile([128, 1], F32)
            nc.sync.dma_start(battn[:], battn_d.ap())
            basey = cpool.tile([128, 576], F32)
            nc.sync.dma_start(basey[:], basey_d.ap())
            basex = cpool.tile([128, 576], F32)
            nc.sync.dma_start(basex[:], basex_d.ap())
            iota = cpool.tile([128, 1], F32)
            nc.sync.dma_start(iota[:], iota_d.ap())
            ident = cpool.tile([128, 128], F32)
            nc.sync.dma_start(ident[:], ident_d.ap())
            identb = cpool.tile([128, 128], mybir.dt.bfloat16)
            nc.vector.tensor_copy(identb[:], ident[:])

            # ---- 1. offset conv: offT [128 x, 64 y, 18 ch] ----
            offT = mpool.tile([128, 64, 18], F32)
            for y in range(HALF):
                po = psum.tile([128, 18], F32, tag="sm")
                for s in range(9):
                    sy, sx = s // 3, s % 3
                    lhsT = xc[:, (y + sy) * PADR + sx:(y + sy) * PADR + sx + 128]
                    nc.tensor.matmul(po[:], lhsT, woff[:, s * 18:(s + 1) * 18],
                                     start=(s == 0), stop=(s == 8))
                nc.vector.tensor_tensor(offT[:, y, :], po[:], boff[:], ALU.add)
            if debug:
                nc.sync.dma_start(
                    dbg_off_d.ap(),
                    offT[:].rearrange("p a b -> p (a b)"))

            # ---- 2. map math ([128 x, 64 y, 9 k] each) ----
            offv = offT[:].rearrange("p y (k two) -> p y k two", two=2)
            dy = offv[:, :, :, 0]   # [128,64,9] stride-2 view
            dx = offv[:, :, :, 1]
            ysb = basey[:].rearrange("p (y k) -> p y k", k=9)
            xsb = basex[:].rearrange("p (y k) -> p y k", k=9)

            ys = mpool.tile([128, 64, 9], F32)
            xs = mpool.tile([128, 64, 9], F32)
            nc.vector.tensor_tensor(ys[:], dy, ysb, ALU.add)
            nc.vector.tensor_tensor(xs[:], dx, xsb, ALU.add)
            # xs += x (iota per partition)
            nc.vector.tensor_scalar(xs[:], xs[:], iota[:], None, ALU.add)

            fy = mpool.tile([128, 64, 9], F32)
            fx = mpool.tile([128, 64, 9], F32)
            y0 = mpool.tile([128, 64, 9], F32)
            x0 = mpool.tile([128, 64, 9], F32)
            yi = tpool.tile([128, 64, 9], mybir.dt.int32, tag="yi")
            xi = tpool.tile([128, 64, 9], mybir.dt.int32, tag="xi")
            corr = tpool.tile([128, 64, 9], F32, tag="corr")
            nc.vector.tensor_copy(yi[:], ys[:])
            nc.vector.tensor_copy(y0[:], yi[:])
            nc.vector.tensor_tensor(corr[:], y0[:], ys[:], ALU.is_gt)
            nc.vector.tensor_tensor(y0[:], y0[:], corr[:], ALU.subtract)
            nc.vector.tensor_tensor(fy[:], ys[:], y0[:], ALU.subtract)
            nc.vector.tensor_copy(xi[:], xs[:])
            nc.vector.tensor_copy(x0[:], xi[:])
            nc.vector.tensor_tensor(corr[:], x0[:], xs[:], ALU.is_gt)
            nc.vector.tensor_tensor(x0[:], x0[:], corr[:], ALU.subtract)
            nc.vector.tensor_tensor(fx[:], xs[:], x0[:], ALU.subtract)

            mB = tpool.tile([128, 64, 9], F32, tag="mB")
            mR = tpool.tile([128, 64, 9], F32, tag="mR")
            nc.vector.tensor_scalar(mB[:], y0[:], -1.0, None, ALU.is_ge)
            nc.vector.tensor_scalar(mR[:], x0[:], -1.0, None, ALU.is_ge)

            # clamp in place
            nc.vector.tensor_scalar(y0[:], y0[:], -1.0, 128.0, ALU.max, ALU.min)
            nc.vector.tensor_scalar(x0[:], x0[:], -1.0, 128.0, ALU.max, ALU.min)

            idxf = mpool.tile([128, 64, 9], F32)
            nc.vector.tensor_scalar(idxf[:], y0[:], 130.0, 131.0,
                                    ALU.mult, ALU.add)
            nc.vector.tensor_tensor(idxf[:], idxf[:], x0[:], ALU.add)

            # weights
            wt = tpool.tile([128, 64, 9], F32, tag="wt")
            wb = tpool.tile([128, 64, 9], F32, tag="wb")
            gx = tpool.tile([128, 64, 9], F32, tag="gx")
            hx = tpool.tile([128, 64, 9], F32, tag="hx")
            nc.vector.tensor_scalar(wt[:], fy[:], -1.0, 1.0, ALU.mult, ALU.add)
            nc.vector.tensor_tensor(wb[:], fy[:], mB[:], ALU.mult)
            nc.vector.tensor_scalar(gx[:], fx[:], -1.0, 1.0, ALU.mult, ALU.add)
            nc.vector.tensor_tensor(hx[:], fx[:], mR[:], ALU.mult)
            w00 = mpool.tile([128, 64, 9], F32)
            w01 = mpool.tile([128, 64, 9], F32)
            w10 = mpool.tile([128, 64, 9], F32)
            w11 = mpool.tile([128, 64, 9], F32)
            nc.vector.tensor_tensor(w00[:], wt[:], gx[:], ALU.mult)
            nc.vector.tensor_tensor(w01[:], wt[:], hx[:], ALU.mult)
            nc.vector.tensor_tensor(w10[:], wb[:], gx[:], ALU.mult)
            nc.vector.tensor_tensor(w11[:], wb[:], hx[:], ALU.mult)
            if debug:
                for i, wm in enumerate([w00, w01, w10, w11]):
                    nc.sync.dma_start(
                        dbg_w_d.ap()[:, i * 576:(i + 1) * 576],
                        wm[:].rearrange("p a b -> p (a b)"))

            # ---- 3. idx relayout -> wrapped [16(+rep), 9k*64y*8xh] int16 ----
            wrapped = mpool.tile([128, 9 * 512], I16)
            for k in range(9):
                p1 = psum.tile([64, 128], F32, tag="sm")
                nc.tensor.transpose(p1[:], idxf[:, :, k], ident[:])
                sb1 = tpool.tile([64, 128], F32, tag="relay_sb")
                nc.scalar.activation(sb1[:], p1[:], AF.Copy)
                for xh in range(8):
                    p2 = psum.tile([16, 64], F32, tag="sm")
                    nc.tensor.transpose(p2[:], sb1[:, xh * 16:(xh + 1) * 16],
                                        ident[0:64, 0:64])
                    wv = wrapped[:].rearrange(
                        "p (k y x) -> p k y x", k=9, y=64)
                    nc.vector.tensor_copy(wv[0:16, k, :, xh], p2[:])
            for g in range(1, 8):
                nc.sync.dma_start(wrapped[16 * g:16 * (g + 1), :],
                                  wrapped[0:16, :])
            if debug:
                nc.sync.dma_start(dbg_idx_d.ap(), wrapped[:])

            # ---- 4-7. gather / blend / conv / gate, per yb block ----
            gbufs = [spool.tile([128, YB, ELEM], F32,
                                 tag=f"gbuf{i}", name=f"gbuf{i}", bufs=1)
                     for i in range(3)]
            gcnt = [0, 0, 0]
            s_all = spool.tile([128, 9, YB, 128], mybir.dt.bfloat16,
                               tag="s_all", name="s_all", bufs=1)
            ci = 0
            for yb in range(NCHUNK):
                obig = [opsum.tile([128, 512], F32, tag="ob",
                                    name=f"ob{yb}_{i}")
                        for i in range(2)]
                outps = [obig[y // 4][:, (y % 4) * 128:(y % 4) * 128 + 128]
                         for y in range(YB)]
                for kg in range(3):   # groups of 3 kernel points
                    for k in range(kg * 3, kg * 3 + 3):
                        bi = k % 3
                        idxs = wrapped[:, k * 512 + yb * 64:
                                       k * 512 + yb * 64 + 64]
                        nc.gpsimd.dma_gather(
                            gbufs[bi][:], pair_ap, idxs, GS, GS, ELEM,
                            elem_step=ESTEP,
                        ).then_inc(gsems[bi], 16)
                        gcnt[bi] += 1
                    for k in range(kg * 3, kg * 3 + 3):
                        bi = k % 3
                        g = gbufs[bi]
                        gv = g[:].rearrange("p y (h r c) -> p y h r c",
                                            h=2, r=2)
                        acc = spool.tile([128, YB, 128], mybir.dt.bfloat16,
                                         tag="acc", bufs=2)
                        tmp = spool.tile([128, YB, 128], mybir.dt.bfloat16,
                                         tag="btmp", bufs=2)
                        corners = [(0, 0, w00), (0, 1, w01), (1, 0, w10),
                                   (1, 1, w11)]
                        for nci_, (r, h, wm) in enumerate(corners):
                            wsl = _bcast0(
                                wm[:, yb * YB:(yb + 1) * YB, k], 128)
                            dst = acc if nci_ == 0 else tmp
                            nc.vector.tensor_tensor(
                                dst[:], gv[:, :, h, r, :], wsl, ALU.mult,
                            )._wait_ge(gsems[bi], 16 * gcnt[bi])
                            if nci_ > 0:
                                nc.vector.tensor_tensor(
                                    acc[:], acc[:], tmp[:], ALU.add)
                        if debug and yb == 0:
                            nc.sync.dma_start(
                                dbg_s_d.ap()[:, k * 128:(k + 1) * 128],
                                acc[:, 0, :])
                        for y in range(YB):
                            ptr = psum.tile([128, 128], mybir.dt.bfloat16,
                                            tag="smb",
                                            name=f"ptr{yb}_{k}_{y}")
                            nc.tensor.transpose(ptr[:], acc[:, y, :],
                                                identb[:])
                            nc.scalar.activation(s_all[:, k, y, :], ptr[:],
                                                 AF.Copy)
                # conv: per y, 9 contiguous PSUM-accumulated matmuls
                for y in range(YB):
                    for k in range(9):
                        nc.tensor.matmul(outps[y], s_all[:, k, y, :],
                                         wdef[:, k * 128:(k + 1) * 128],
                                         start=(k == 0), stop=(k == 8))
                # post (batched per chunk): bias, attention gate, relu, store
                outf = spool.tile([128, YB, 128], F32, tag="outf", bufs=2)
                for i in range(2):
                    nc.vector.tensor_tensor(
                        outf[:, i * 4:(i + 1) * 4, :],
                        obig[i][:].rearrange("p (a b) -> p a b", a=4),
                        bdef_rep4, ALU.add)
                tmpo = spool.tile([128, YB, 128], F32, tag="tmpo", bufs=2)
                nc.vector.tensor_tensor(
                    tmpo[:], outf[:], _bcast_tile(wattn, YB), ALU.mult)
                attn = spool.tile([128, YB, 1], F32, tag="attn", bufs=2)
                nc.vector.reduce_sum(attn[:], tmpo[:],
                                     axis=mybir.AxisListType.X)
                sig = spool.tile([128, YB], F32, tag="sig", bufs=2)
                nc.scalar.activation(sig[:], attn[:, :, 0], AF.Sigmoid,
                                     bias=battn[:])
                fin = spool.tile([128, YB, 128], F32, tag="fin", bufs=2)
                nc.vector.tensor_tensor(
                    fin[:], outf[:], _bcast0(sig[:], 128), ALU.mult)
                nc.vector.tensor_scalar(
                    fin[:], fin[:], 0.0, None, ALU.max)
                nc.sync.dma_start(
                    out_d.ap()[:, yb * YB * 128:(yb + 1) * YB * 128],
                    fin[:].rearrange("p a b -> p (a b)"))

    nc.compile()
    return nc


def _prep_inputs(x, w_off, b_off, w_def, b_def, w_attn, b_attn):
    """Host-side layout marshalling (pure reshape/transpose/pad/cast)."""
    x = np.asarray(x, dtype=np.float32)
    in_maps = []
    # pair image per batch: rows (y,x) padded, entry = [x[y,x,:], x[y+1,x,:]]
    pairs = []
    for b in range(B):
        xp = np.zeros((PADR + 1, PADR, C), dtype=np.float32)  # y in [-1,129]
        xp[1:H + 1, 1:W + 1, :] = x[b].transpose(1, 2, 0)
        pair = np.zeros((NPAIR, ESTEP), dtype=np.float32)
        ent = np.concatenate([xp[:PADR], xp[1:PADR + 1]], axis=2)
        pair[:PADR * PADR] = ent.reshape(PADR * PADR, ESTEP)
        pairs.append(pair.reshape(-1))

    woffT = np.zeros((128, 9 * 18), dtype=np.float32)
    for s in range(9):
        woffT[:, s * 18:(s + 1) * 18] = w_off[:, :, s // 3, s % 3].T
    boff_rep = np.broadcast_to(b_off[None, :], (128, 18)).copy().astype(np.float32)
    import ml_dtypes
    wdefT = np.zeros((128, 9 * 128), dtype=ml_dtypes.bfloat16)
    wdr = w_def.reshape(C, C, 9)
    for k in range(9):
        wdefT[:, k * 128:(k + 1) * 128] = wdr[:, :, k].T.astype(
            ml_dtypes.bfloat16)  # [c, o]
    bdef_rep = np.broadcast_to(b_def[None, :], (128, 128)).copy().astype(np.float32)
    wattn_rep = np.broadcast_to(w_attn[:, :, 0, 0].reshape(1, C),
                                (128, C)).copy().astype(np.float32)
    battn_rep = np.full((128, 1), float(b_attn[0]), dtype=np.float32)
    iota = np.arange(128, dtype=np.float32).reshape(128, 1)
    ident = np.eye(128, dtype=np.float32)
    ky, kx = np.meshgrid(np.arange(3), np.arange(3), indexing="ij")
    ky = ky.reshape(9).astype(np.float32)
    kx = kx.reshape(9).astype(np.float32)
    basex = np.broadcast_to((kx - 1.0)[None, None, :],
                            (128, 64, 9)).reshape(128, 576).copy()

    for core in range(N_CORES):
        b, h = core // 2, core % 2
        yg = 64 * h + np.arange(64, dtype=np.float32)
        basey = np.broadcast_to(
            (yg[:, None] - 1.0 + ky[None, :])[None], (128, 64, 9),
        ).reshape(128, 576).copy()
        # c-major padded halo slice [128, 66, 130]
        xcp = np.zeros((128, 66, PADR), dtype=np.float32)
        r0 = 64 * h - 1
        for r in range(66):
            yy = r0 + r
            if 0 <= yy < H:
                xcp[:, r, 1:W + 1] = x[b, :, yy, :]
        in_maps.append({
            "pair": pairs[b],
            "xc": xcp.reshape(128, 66 * PADR),
            "woff": woffT, "boff": boff_rep,
            "wdef": wdefT, "bdef": bdef_rep,
            "wattn": wattn_rep, "battn": battn_rep,
            "basey": basey.astype(np.float32),
            "basex": basex.astype(np.float32),
            "iota": iota, "ident": ident,
        })
    return in_maps


def kernel(x, w_off, b_off, w_def, b_def, w_attn, b_attn, _debug=False,
           _trace=False):
    key = ("prog", _debug)
    if key not in _cache:
        _cache[key] = build_program(debug=_debug)
    nc = _cache[key]
    in_maps = _prep_inputs(np.asarray(x), np.asarray(w_off),
                           np.asarray(b_off), np.asarray(w_def),
                           np.asarray(b_def), np.asarray(w_attn),
                           np.asarray(b_attn))
    res = run_bass_kernel_spmd(nc, in_maps, list(range(N_CORES)),
                               trace=_trace)
    out = np.zeros((B, C, H, W), dtype=np.float32)
    for core in range(N_CORES):
        b, h = core // 2, core % 2
        o = res.results[core]["out"].reshape(128, 64, 128)  # [x, y, o]
        out[b, :, 64 * h:64 * h + 64, :] = o.transpose(2, 1, 0)
    kernel._last = res
    return out


# revision 22
# speedup vs baseline: 1.2677x; 1.2677x over previous
"""Deformable-attention block kernel for Trainium2 (8 NeuronCores).

Sharding: data-parallel over (batch, image-half): core i handles image i//2,
rows [64*(i%2), 64*(i%2)+64).

Per-core pipeline (all compute on device):
  1. offset conv (3x3, PE f32, PSUM-accumulated shifted matmuls)
  2. per-pixel map math (DVE f32): sample coords, floor/frac, clamps, masks,
     bilinear corner weights, flat gather indices
  3. index relayout to dma_gather's wrapped [16, S] layout via PE transposes
  4. dma_gather from a padded row-pair image in HBM: ONE descriptor per
     (kernel-point, pixel) fetches the full 2x2 bilinear patch
  5. bilinear blend (DVE, per-partition weights with stride-0 broadcast)
  6. PE transpose + deform conv (9 matmuls accumulated in PSUM)
  7. bias + sigmoid attention gate + relu, pixel-major output
"""
import dataclasses
import sys

import numpy as np

sys.path.insert(0, "/opt/trn_rl_repo")

import concourse.bacc as bacc  # noqa: E402
import concourse.bass as bass  # noqa: E402
import concourse.mybir as mybir  # noqa: E402
import concourse.tile as tile  # noqa: E402
from concourse.bass_utils import run_bass_kernel_spmd  # noqa: E402
from concourse.library_config import mlp  # noqa: E402

F32 = mybir.dt.float32
I16 = mybir.dt.int16
AF = mybir.ActivationFunctionType
ALU = mybir.AluOpType

B, C, H, W = 4, 128, 128, 128
N_CORES = 8
HALF = 64          # output rows per core
PADR = 130         # padded coords: y,x in [-1,128]
NPAIR = PADR * PADR + 8   # pair-image rows (+ slack zeros for idx+1 overflow)
ELEM = 512         # f32 elements per gather descriptor (2 pair-entries)
ESTEP = 256        # f32 elements per pair-entry row
YB = 8             # output rows per gather chunk
NCHUNK = HALF // YB  # 8 chunks per k
GS = 1024          # idxs per gather instruction (8 rows x 128 px)

_cache = {}


def _bcast0(ap, n):
    """Append a stride-0 dim of size n to an AP (free-dim broadcast)."""
    return dataclasses.replace(ap, ap=list(ap.ap) + [[0, n]])


def _bcast_tile(t, n):
    """[128, N] tile -> [128, n, N] with stride-0 middle dim."""
    ap = t[:]
    return dataclasses.replace(
        ap, ap=[ap.ap[0], [0, n], ap.ap[1]])


def build_program(debug=False):
    nc = bacc.Bacc("TRN2", target_bir_lowering=False, debug=False,
                   num_devices=N_CORES)

    # ---- DRAM I/O ----
    pair_d = nc.dram_tensor("pair", [NPAIR * ESTEP], F32, kind="ExternalInput")
    pair_ap = dataclasses.replace(
        pair_d.ap(), ap=[[ESTEP, NPAIR - 1], [1, ELEM]])
    xc_d = nc.dram_tensor("xc", [128, 66 * PADR], F32, kind="ExternalInput")
    woff_d = nc.dram_tensor("woff", [128, 9 * 18], F32, kind="ExternalInput")
    boff_d = nc.dram_tensor("boff", [128, 18], F32, kind="ExternalInput")
    wdef_d = nc.dram_tensor("wdef", [128, 9 * 128], mybir.dt.bfloat16, kind="ExternalInput")
    bdef_d = nc.dram_tensor("bdef", [128, 128], F32, kind="ExternalInput")
    wattn_d = nc.dram_tensor("wattn", [128, 128], F32, kind="ExternalInput")
    battn_d = nc.dram_tensor("battn", [128, 1], F32, kind="ExternalInput")
    basey_d = nc.dram_tensor("basey", [128, 576], F32, kind="ExternalInput")
    basex_d = nc.dram_tensor("basex", [128, 576], F32, kind="ExternalInput")
    iota_d = nc.dram_tensor("iota", [128, 1], F32, kind="ExternalInput")
    ident_d = nc.dram_tensor("ident", [128, 128], F32, kind="ExternalInput")
    out_d = nc.dram_tensor("out", [128, HALF * 128], F32, kind="ExternalOutput")
    if debug:
        dbg_off_d = nc.dram_tensor("dbg_off", [128, 64 * 18], F32,
                                   kind="ExternalOutput")
        dbg_idx_d = nc.dram_tensor("dbg_idx", [128, 4608], I16,
                                   kind="ExternalOutput")
        dbg_w_d = nc.dram_tensor("dbg_w", [128, 4 * 576], F32,
                                 kind="ExternalOutput")
        dbg_s_d = nc.dram_tensor("dbg_s", [128, 9 * 128],
                                 mybir.dt.bfloat16, kind="ExternalOutput")

    gsems = [nc.alloc_semaphore(f"gsem{i}") for i in range(4)]

    with tile.TileContext(nc) as tc:
        import contextlib
        with contextlib.ExitStack() as ctx:
            cpool = ctx.enter_context(tc.tile_pool(name="consts", bufs=1))
            mpool = ctx.enter_context(tc.tile_pool(name="maps", bufs=1))
            tpool = ctx.enter_context(tc.tile_pool(name="tmp", bufs=1))
            spool = ctx.enter_context(tc.tile_pool(name="swork", bufs=3))
            psum = ctx.enter_context(
                tc.tile_pool(name="psum", bufs=2, space="PSUM"))
            opsum = ctx.enter_context(
                tc.tile_pool(name="opsum", bufs=4, space="PSUM"))

            # ---- load constants ----
            xc = cpool.tile([128, 66 * PADR], F32)
            nc.sync.dma_start(xc[:], xc_d.ap())
            woff = cpool.tile([128, 9 * 18], F32)
            nc.sync.dma_start(woff[:], woff_d.ap())
            boff = cpool.tile([128, 18], F32)
            nc.sync.dma_start(boff[:], boff_d.ap())
            wdef = cpool.tile([128, 9 * 128], mybir.dt.bfloat16)
            nc.sync.dma_start(wdef[:], wdef_d.ap())
            bdef = cpool.tile([128, 128], F32)
            nc.sync.dma_start(bdef[:], bdef_d.ap())
            bdef_rep4 = dataclasses.replace(
                bdef[:], ap=[bdef[:].ap[0], [0, 4], bdef[:].ap[1]])
            wattn = cpool.tile([128, 128], F32)
            nc.sync.dma_start(wattn[:], wattn_d.ap())
            battn = cpool.tile([128, 1], F32)
            nc.sync.dma_start(battn[:], battn_d.ap())
            basey = cpool.tile([128, 576], F32)
            nc.sync.dma_start(basey[:], basey_d.ap())
            basex = cpool.tile([128, 576], F32)
            nc.sync.dma_start(basex[:], basex_d.ap())
            iota = cpool.tile([128, 1], F32)
            nc.sync.dma_start(iota[:], iota_d.ap())
            ident = cpool.tile([128, 128], F32)
            nc.sync.dma_start(ident[:], ident_d.ap())
            identb = cpool.tile([128, 128], mybir.dt.bfloat16)
            nc.vector.tensor_copy(identb[:], ident[:])

            # ---- 1. offset conv: offT [128 x, 64 y, 18 ch] ----
            offT = mpool.tile([128, 64, 18], F32)
            for y in range(HALF):
                po = psum.tile([128, 18], F32, tag="sm")
                for s in range(9):
                    sy, sx = s // 3, s % 3
                    lhsT = xc[:, (y + sy) * PADR + sx:(y + sy) * PADR + sx + 128]
                    nc.tensor.matmul(po[:], lhsT, woff[:, s * 18:(s + 1) * 18],
                                     start=(s == 0), stop=(s == 8))
                nc.vector.tensor_tensor(offT[:, y, :], po[:], boff[:], ALU.add)
            if debug:
                nc.sync.dma_start(
                    dbg_off_d.ap(),
                    offT[:].rearrange("p a b -> p (a b)"))

            # ---- 2. map math ([128 x, 64 y, 9 k] each) ----
            offv = offT[:].rearrange("p y (k two) -> p y k two", two=2)
            dy = offv[:, :, :, 0]   # [128,64,9] stride-2 view
            dx = offv[:, :, :, 1]
            ysb = basey[:].rearrange("p (y k) -> p y k", k=9)
            xsb = basex[:].rearrange("p (y k) -> p y k", k=9)

            ys = mpool.tile([128, 64, 9], F32)
            xs = mpool.tile([128, 64, 9], F32)
            nc.vector.tensor_tensor(ys[:], dy, ysb, ALU.add)
            nc.vector.tensor_tensor(xs[:], dx, xsb, ALU.add)
            # xs += x (iota per partition)
            nc.vector.tensor_scalar(xs[:], xs[:], iota[:], None, ALU.add)

            fy = mpool.tile([128, 64, 9], F32)
            fx = mpool.tile([128, 64, 9], F32)
            y0 = mpool.tile([128, 64, 9], F32)
            x0 = mpool.tile([128, 64, 9], F32)
            yi = tpool.tile([128, 64, 9], mybir.dt.int32, tag="yi")
            xi = tpool.tile([128, 64, 9], mybir.dt.int32, tag="xi")
            corr = tpool.tile([128, 64, 9], F32, tag="corr")
            nc.vector.tensor_copy(yi[:], ys[:])
            nc.vector.tensor_copy(y0[:], yi[:])
            nc.vector.tensor_tensor(corr[:], y0[:], ys[:], ALU.is_gt)
            nc.vector.tensor_tensor(y0[:], y0[:], corr[:], ALU.subtract)
            nc.vector.tensor_tensor(fy[:], ys[:], y0[:], ALU.subtract)
            nc.vector.tensor_copy(xi[:], xs[:])
            nc.vector.tensor_copy(x0[:], xi[:])
            nc.vector.tensor_tensor(corr[:], x0[:], xs[:], ALU.is_gt)
            nc.vector.tensor_tensor(x0[:], x0[:], corr[:], ALU.subtract)
            nc.vector.tensor_tensor(fx[:], xs[:], x0[:], ALU.subtract)

            mB = tpool.tile([128, 64, 9], F32, tag="mB")
            mR = tpool.tile([128, 64, 9], F32, tag="mR")
            nc.vector.tensor_scalar(mB[:], y0[:], -1.0, None, ALU.is_ge)
            nc.vector.tensor_scalar(mR[:], x0[:], -1.0, None, ALU.is_ge)

            # clamp in place
            nc.vector.tensor_scalar(y0[:], y0[:], -1.0, 128.0, ALU.max, ALU.min)
            nc.vector.tensor_scalar(x0[:], x0[:], -1.0, 128.0, ALU.max, ALU.min)

            idxf = mpool.tile([128, 64, 9], F32)
            nc.vector.tensor_scalar(idxf[:], y0[:], 130.0, 131.0,
                                    ALU.mult, ALU.add)
            nc.vector.tensor_tensor(idxf[:], idxf[:], x0[:], ALU.add)

            # weights
            wt = tpool.tile([128, 64, 9], F32, tag="wt")
            wb = tpool.tile([128, 64, 9], F32, tag="wb")
            gx = tpool.tile([128, 64, 9], F32, tag="gx")
            hx = tpool.tile([128, 64, 9], F32, tag="hx")
            nc.vector.tensor_scalar(wt[:], fy[:], -1.0, 1.0, ALU.mult, ALU.add)
            nc.vector.tensor_tensor(wb[:], fy[:], mB[:], ALU.mult)
            nc.vector.tensor_scalar(gx[:], fx[:], -1.0, 1.0, ALU.mult, ALU.add)
            nc.vector.tensor_tensor(hx[:], fx[:], mR[:], ALU.mult)
            w00 = mpool.tile([128, 64, 9], F32)
            w01 = mpool.tile([128, 64, 9], F32)
            w10 = mpool.tile([128, 64, 9], F32)
            w11 = mpool.tile([128, 64, 9], F32)
            nc.vector.tensor_tensor(w00[:], wt[:], gx[:], ALU.mult)
            nc.vector.tensor_tensor(w01[:], wt[:], hx[:], ALU.mult)
            nc.vector.tensor_tensor(w10[:], wb[:], gx[:], ALU.mult)
            nc.vector.tensor_tensor(w11[:], wb[:], hx[:], ALU.mult)
            if debug:
                for i, wm in enumerate([w00, w01, w10, w11]):
                    nc.sync.dma_start(
                        dbg_w_d.ap()[:, i * 576:(i + 1) * 576],
                        wm[:].rearrange("p a b -> p (a b)"))

            # ---- 3. idx relayout -> wrapped [16(+rep), 9k*64y*8xh] int16 ----
            wrapped = mpool.tile([128, 9 * 512], I16)
            for k in range(9):
                p1 = psum.tile([64, 128], F32, tag="sm")
                nc.tensor.transpose(p1[:], idxf[:, :, k], ident[:])
                sb1 = tpool.tile([64, 128], F32, tag="relay_sb")
                nc.scalar.activation(sb1[:], p1[:], AF.Copy)
                for xh in range(8):
                    p2 = psum.tile([16, 64], F32, tag="sm")
                    nc.tensor.transpose(p2[:], sb1[:, xh * 16:(xh + 1) * 16],
                                        ident[0:64, 0:64])
                    wv = wrapped[:].rearrange(
                        "p (k y x) -> p k y x", k=9, y=64)
                    nc.vector.tensor_copy(wv[0:16, k, :, xh], p2[:])
            for g in range(1, 8):
                nc.sync.dma_start(wrapped[16 * g:16 * (g + 1), :],
                                  wrapped[0:16, :])
            if debug:
                nc.sync.dma_start(dbg_idx_d.ap(), wrapped[:])

            # ---- 4-7. gather / blend / conv / gate, per yb block ----
            gbufs = [spool.tile([128, YB, ELEM], F32,
                                 tag=f"gbuf{i}", name=f"gbuf{i}", bufs=1)
                     for i in range(4)]
            gcnt = [0, 0, 0, 0]
            NB = 4

            def gspec(gk):
                return gk // 9, gk % 9   # (yb, k)

            gtarget = {}

            def issue_gather(gk):
                yb_, k_ = gspec(gk)
                bi_ = gk % NB
                idxs_ = wrapped[:, k_ * 512 + yb_ * 64:
                                k_ * 512 + yb_ * 64 + 64]
                nc.gpsimd.dma_gather(
                    gbufs[bi_][:], pair_ap, idxs_, GS, GS, ELEM,
                    elem_step=ESTEP,
                ).then_inc(gsems[bi_], 16)
                gcnt[bi_] += 1
                gtarget[gk] = 16 * gcnt[bi_]

            for gk in range(NB):
                issue_gather(gk)
            s_all = spool.tile([128, 9, YB, 128], mybir.dt.bfloat16,
                               tag="s_all", name="s_all", bufs=1)
            ci = 0
            for yb in range(NCHUNK):
                obig = [opsum.tile([128, 512], F32, tag="ob",
                                    name=f"ob{yb}_{i}")
                        for i in range(2)]
                outps = [obig[y // 4][:, (y % 4) * 128:(y % 4) * 128 + 128]
                         for y in range(YB)]
                if True:
                    for k in range(9):
                        gk = yb * 9 + k
                        bi = gk % NB
                        g = gbufs[bi]
                        gv = g[:].rearrange("p y (h r c) -> p y h r c",
                                            h=2, r=2)
                        acc = spool.tile([128, YB, 128], mybir.dt.bfloat16,
                                         tag="acc", bufs=2)
                        tmp = spool.tile([128, YB, 128], mybir.dt.bfloat16,
                                         tag="btmp", bufs=2)
                        corners = [(0, 0, w00), (0, 1, w01), (1, 0, w10),
                                   (1, 1, w11)]
                        for nci_, (r, h, wm) in enumerate(corners):
                            wsl = _bcast0(
                                wm[:, yb * YB:(yb + 1) * YB, k], 128)
                            dst = acc if nci_ == 0 else tmp
                            nc.vector.tensor_tensor(
                                dst[:], gv[:, :, h, r, :], wsl, ALU.mult,
                            )._wait_ge(gsems[bi], gtarget[gk])
                            if nci_ > 0:
                                nc.vector.tensor_tensor(
                                    acc[:], acc[:], tmp[:], ALU.add)
                        if yb * 9 + k + NB < 72:
                            issue_gather(yb * 9 + k + NB)
                        if debug and yb == 0:
                            nc.sync.dma_start(
                                dbg_s_d.ap()[:, k * 128:(k + 1) * 128],
                                acc[:, 0, :])
                        for y in range(YB):
                            ptr = psum.tile([128, 128], mybir.dt.bfloat16,
                                            tag="smb",
                                            name=f"ptr{yb}_{k}_{y}")
                            nc.tensor.transpose(ptr[:], acc[:, y, :],
                                                identb[:])
                            nc.scalar.activation(s_all[:, k, y, :], ptr[:],
                                                 AF.Copy)
                # conv: per y, 9 contiguous PSUM-accumulated matmuls
                for y in range(YB):
                    for k in range(9):
                        nc.tensor.matmul(outps[y], s_all[:, k, y, :],
                                         wdef[:, k * 128:(k + 1) * 128],
                                         start=(k == 0), stop=(k == 8))
                # post (batched per chunk): bias, attention gate, relu, store
                outf = spool.tile([128, YB, 128], F32, tag="outf", bufs=1)
                for i in range(2):
                    nc.vector.tensor_tensor(
                        outf[:, i * 4:(i + 1) * 4, :],
                        obig[i][:].rearrange("p (a b) -> p a b", a=4),
                        bdef_rep4, ALU.add)
                tmpo = spool.tile([128, YB, 128], F32, tag="fin", bufs=2)
                nc.vector.tensor_tensor(
                    tmpo[:], outf[:], _bcast_tile(wattn, YB), ALU.mult)
                attn = spool.tile([128, YB, 1], F32, tag="attn", bufs=2)
                nc.vector.reduce_sum(attn[:], tmpo[:],
                                     axis=mybir.AxisListType.X)
                sig = spool.tile([128, YB], F32, tag="sig", bufs=2)
                nc.scalar.activation(sig[:], attn[:, :, 0], AF.Sigmoid,
                                     bias=battn[:])
                fin = spool.tile([128, YB, 128], F32, tag="fin", bufs=2)
                nc.vector.tensor_tensor(
                    fin[:], outf[:], _bcast0(sig[:], 128), ALU.mult)
                nc.vector.tensor_scalar(
                    fin[:], fin[:], 0.0, None, ALU.max)
                nc.sync.dma_start(
                    out_d.ap()[:, yb * YB * 128:(yb + 1) * YB * 128],
                    fin[:].rearrange("p a b -> p (a b)"))

    nc.compile()
    return nc


def _prep_inputs(x, w_off, b_off, w_def, b_def, w_attn, b_attn):
    """Host-side layout marshalling (pure reshape/transpose/pad/cast)."""
    x = np.asarray(x, dtype=np.float32)
    in_maps = []
    # pair image per batch: rows (y,x) padded, entry = [x[y,x,:], x[y+1,x,:]]
    pairs = []
    for b in range(B):
        xp = np.zeros((PADR + 1, PADR, C), dtype=np.float32)  # y in [-1,129]
        xp[1:H + 1, 1:W + 1, :] = x[b].transpose(1, 2, 0)
        pair = np.zeros((NPAIR, ESTEP), dtype=np.float32)
        ent = np.concatenate([xp[:PADR], xp[1:PADR + 1]], axis=2)
        pair[:PADR * PADR] = ent.reshape(PADR * PADR, ESTEP)
        pairs.append(pair.reshape(-1))

    woffT = np.zeros((128, 9 * 18), dtype=np.float32)
    for s in range(9):
        woffT[:, s * 18:(s + 1) * 18] = w_off[:, :, s // 3, s % 3].T
    boff_rep = np.broadcast_to(b_off[None, :], (128, 18)).copy().astype(np.float32)
    import ml_dtypes
    wdefT = np.zeros((128, 9 * 128), dtype=ml_dtypes.bfloat16)
    wdr = w_def.reshape(C, C, 9)
    for k in range(9):
        wdefT[:, k * 128:(k + 1) * 128] = wdr[:, :, k].T.astype(
            ml_dtypes.bfloat16)  # [c, o]
    bdef_rep = np.broadcast_to(b_def[None, :], (128, 128)).copy().astype(np.float32)
    wattn_rep = np.broadcast_to(w_attn[:, :, 0, 0].reshape(1, C),
                                (128, C)).copy().astype(np.float32)
    battn_rep = np.full((128, 1), float(b_attn[0]), dtype=np.float32)
    iota = np.arange(128, dtype=np.float32).reshape(128, 1)
    ident = np.eye(128, dtype=np.float32)
    ky, kx = np.meshgrid(np.arange(3), np.arange(3), indexing="ij")
    ky = ky.reshape(9).astype(np.float32)
    kx = kx.reshape(9).astype(np.float32)
    basex = np.broadcast_to((kx - 1.0)[None, None, :],
                            (128, 64, 9)).reshape(128, 576).copy()

    for core in range(N_CORES):
        b, h = core // 2, core % 2
        yg = 64 * h + np.arange(64, dtype=np.float32)
        basey = np.broadcast_to(
            (yg[:, None] - 1.0 + ky[None, :])[None], (128, 64, 9),
        ).reshape(128, 576).copy()
        # c-major padded halo slice [128, 66, 130]
        xcp = np.zeros((128, 66, PADR), dtype=np.float32)
        r0 = 64 * h - 1
        for r in range(66):
            yy = r0 + r
            if 0 <= yy < H:
                xcp[:, r, 1:W + 1] = x[b, :, yy, :]
        in_maps.append({
            "pair": pairs[b],
            "xc": xcp.reshape(128, 66 * PADR),
            "woff": woffT, "boff": boff_rep,
            "wdef": wdefT, "bdef": bdef_rep,
            "wattn": wattn_rep, "battn": battn_rep,
            "basey": basey.astype(np.float32),
            "basex": basex.astype(np.float32),
            "iota": iota, "ident": ident,
        })
    return in_maps


def kernel(x, w_off, b_off, w_def, b_def, w_attn, b_attn, _debug=False,
           _trace=False):
    key = ("prog", _debug)
    if key not in _cache:
        _cache[key] = build_program(debug=_debug)
    nc = _cache[key]
    in_maps = _prep_inputs(np.asarray(x), np.asarray(w_off),
                           np.asarray(b_off), np.asarray(w_def),
                           np.asarray(b_def), np.asarray(w_attn),
                           np.asarray(b_attn))
    res = run_bass_kernel_spmd(nc, in_maps, list(range(N_CORES)),
                               trace=_trace)
    out = np.zeros((B, C, H, W), dtype=np.float32)
    for core in range(N_CORES):
        b, h = core // 2, core % 2
        o = res.results[core]["out"].reshape(128, 64, 128)  # [x, y, o]
        out[b, :, 64 * h:64 * h + 64, :] = o.transpose(2, 1, 0)
    kernel._last = res
    return out
